# revision 1
# baseline (speedup 1.0000x reference)
"""MinGRU Trainium2 kernel (v3: bf16 matmul, ACT/DVE-balanced pointwise,
split DMA rings).

Full-input contract: kernel(x=[8,4096,1024] f32, W_hg=[2048,1024] f32)
-> [8,4096,1024] f32.

Sharding: data-parallel over batch. Core i computes example i entirely;
W_hg is replicated.

Math (linear-space equivalent of the log-space reference):
    hg      = x @ W_hg.T ; hidden, gate = split(hg)
    a_t     = sigmoid(-gate_t)                       # = 1 - z_t
    g~_t    = max(sigmoid(hidden_t), hidden_t + 0.5) # exact identity
    h_t     = a_t h_{t-1} + (1 - a_t) g~_t
            = a_t h_{t-1} - bneg_t,  bneg_t = (a_t - 1) g~_t

Per (sc, db) tile ([128 channels, 512 seq]):
    PE   : 16 bf16 matmuls -> phg [128, 2, 512] f32 PSUM (2 banks:
           hidden | NEGATED gate; gate weight block negated on host so ONE
           combined sigmoid yields both s and a).
    ACT  : sa = sigmoid(phg)   [128,1024] -> bf16 (s | a)
    ACT  : hp = phg[:,0,:]+0.5 (Copy+bias) -> bf16      [gt_mode="act"]
    DVE  : gt = max(hp, s)     (tensor_tensor, bf16)
           -- or gt = (ph + 0.5) max s via one stt from PSUM [gt_mode="dve"]
    DVE  : bneg = (a - 1.0) * gt                     (stt)
    DVE  : o = scan(a, bneg, init, mult, subtract)   # o_t = a o_{t-1} - bneg
    DMA  : out[db, sc] <- o (bf16; host upcasts)

Loop order: sc OUTER, db inner -> scan carry (db, sc-1) is ready a full
sc block ahead; never on the critical path.

DMA: inputs (W, x) go on the ACT HWDGE ring, outputs on the SP ring, so
input prefetch is not FIFO-blocked behind output drains. x tiles are
reloaded right after their last use each pass (the data is loop-invariant
in the timing loop), so the reload for pass n+1 overlaps ~7/8 of pass n.

bf16 pipeline validated on CPU: Fro rel err ~2.9e-3 (gate: 2e-2).
"""

from contextlib import ExitStack

import numpy as np

B, S, D = 8, 4096, 1024
E = 2 * D
P = 128
KT = D // P  # contraction k-tiles
DB = D // P  # output-channel pair-blocks (hidden+gate pair per block)
SC = 512  # seq chunk (PSUM bank = 512 f32)
NSC = S // SC

_NC_CACHE = {}


def _build_bass(
    repeat=1,
    loop_repeat=None,
    psum_bufs=4,
    sa_bufs=4,
    work_bufs=4,
    gt_mode="act",  # "act": hp on ACT + max on DVE; "dve": stt on DVE
    stages=5,  # ablation: 0=PE only, 1=+sig, 2=+hp, 3=+gt, 4=+bneg, 5=full
    stream_inputs=1,  # 0: prologue-load only (ablation); 1: reload per pass
    mm_order="db",  # "db": per-pair k-inner; "k": k-outer, 2 pairs/group
    x_flat=0,  # 1: x as 64 flat [P,SC] tiles instead of 8 [P,KT,SC]
):
    import contextlib

    import concourse.tile as tile
    from concourse import bacc, mybir

    f32 = mybir.dt.float32
    bf16 = mybir.dt.bfloat16
    AF = mybir.ActivationFunctionType
    OP = mybir.AluOpType

    nc = bacc.Bacc("TRN2", debug=False)
    xT = nc.dram_tensor("xT", [D, S], bf16, kind="ExternalInput").ap()
    wT = nc.dram_tensor("wT", [D, E], bf16, kind="ExternalInput").ap()
    out = nc.dram_tensor("out", [D, S], bf16, kind="ExternalOutput").ap()

    xT_k = xT.rearrange("(k p) s -> p k s", p=P)
    wT_k = wT.rearrange("(k p) e -> p k e", p=P)

    with tile.TileContext(nc) as tc, ExitStack() as ctx:
        xpool = ctx.enter_context(tc.tile_pool(name="x", bufs=1))
        wpool = ctx.enter_context(tc.tile_pool(name="w", bufs=2))
        ppool = ctx.enter_context(
            tc.tile_pool(name="ps", bufs=psum_bufs, space="PSUM")
        )
        sapool = ctx.enter_context(tc.tile_pool(name="sa", bufs=sa_bufs))
        gpool = ctx.enter_context(tc.tile_pool(name="g", bufs=work_bufs))
        opool = ctx.enter_context(tc.tile_pool(name="o", bufs=2))

        # x tiles persist across For_i iterations; prologue-loaded once,
        # then re-loaded (same loop-invariant data) right after last use.
        xt = []
        if x_flat:
            for sc in range(NSC):
                row = []
                for k in range(KT):
                    t = xpool.tile(
                        [P, SC], bf16, tag=f"x{sc}_{k}", name=f"xt{sc}_{k}"
                    )
                    nc.scalar.dma_start(
                        t[:], xT_k[:, k, sc * SC : (sc + 1) * SC]
                    )
                    row.append(t)
                xt.append(row)

            def xs(sc, k):
                return xt[sc][k][:]
        else:
            for sc in range(NSC):
                t = xpool.tile(
                    [P, KT, SC], bf16, tag=f"x{sc}", name=f"xt{sc}"
                )
                nc.scalar.dma_start(
                    t[:], xT_k[:, :, sc * SC : (sc + 1) * SC]
                )
                xt.append(t)

            def xs(sc, k):
                return xt[sc][:, k, :]

        loop_cm = (
            tc.For_i(0, loop_repeat, 1)
            if loop_repeat is not None
            else contextlib.nullcontext()
        )
        # W is persistent (one buffer); prologue-loaded monolithically, then
        # per-pass re-streamed in per-block slices right after each block's
        # last use, so no pass-head 4MB serialization.
        wt = wpool.tile([P, KT, E], bf16, tag="w")
        nc.scalar.dma_start(wt[:], wT_k)

        with loop_cm:
            for _rep in range(repeat):

                def pointwise(sc, db, phg, prev_o):
                    if stages < 1:
                        return
                    # one sigmoid over both banks: s | a
                    sa = sapool.tile([P, 2, SC], bf16, tag="sa", name="sa")
                    nc.scalar.activation(sa[:], phg[:], AF.Sigmoid)
                    s = sa[:, 0, :]
                    a = sa[:, 1, :]

                    gt = gpool.tile([P, SC], bf16, tag="gt", name="gt")
                    if gt_mode == "act":
                        if stages < 2:
                            return
                        hp = gpool.tile([P, SC], bf16, tag="hp", name="hp")
                        nc.scalar.activation(
                            hp[:], phg[:, 0, :], AF.Copy, bias=0.5
                        )
                        if stages < 3:
                            return
                        nc.vector.tensor_tensor(
                            gt[:], hp[:], s, op=OP.max
                        )
                    else:
                        if stages < 3:
                            return
                        nc.vector.scalar_tensor_tensor(
                            gt[:], phg[:, 0, :], 0.5, s,
                            op0=OP.add, op1=OP.max,
                        )

                    if stages < 4:
                        return
                    bneg = gpool.tile([P, SC], bf16, tag="bneg", name="bneg")
                    nc.vector.scalar_tensor_tensor(
                        bneg[:], a, 1.0, gt[:],
                        op0=OP.subtract, op1=OP.mult,
                    )

                    if stages < 5:
                        return
                    o = opool.tile([P, SC], bf16, tag=f"o{db}", name="o")
                    init = (
                        0.0 if sc == 0
                        else prev_o[db][:, SC - 1 : SC]
                    )
                    nc.vector.tensor_tensor_scan(
                        o[:], a, bneg[:], init,
                        op0=OP.mult, op1=OP.subtract,
                    )
                    prev_o[db] = o
                    nc.sync.dma_start(
                        out[db * P : db * P + P, sc * SC : (sc + 1) * SC],
                        o[:],
                    )

                def reload_w(db):
                    # re-stream this block's weight slices (same data) for
                    # the next pass, right after their last use
                    if not stream_inputs:
                        return
                    for base in (db * P, D + db * P):
                        nc.scalar.dma_start(
                            wt[:, :, base : base + P],
                            wT_k[:, :, base : base + P],
                        )

                prev_o = [None] * DB
                for sc in range(NSC):
                    if mm_order in ("db", "pair"):
                        for db in range(DB):
                            eh = db * P
                            eg = D + db * P
                            phg = ppool.tile([P, 2, SC], f32, tag="phg")
                            if mm_order == "pair":
                                # interleave: each x k-slice feeds 2
                                # back-to-back MMs (hidden then gate)
                                for k in range(KT):
                                    for half, base in (
                                        (0, eh), (1, eg),
                                    ):
                                        nc.tensor.matmul(
                                            phg[:, half, :],
                                            wt[:, k, base : base + P],
                                            xs(sc, k),
                                            start=(k == 0),
                                            stop=(k == KT - 1),
                                        )
                            else:
                                for k in range(KT):
                                    nc.tensor.matmul(
                                        phg[:, 0, :],
                                        wt[:, k, eh : eh + P],
                                        xs(sc, k),
                                        start=(k == 0),
                                        stop=(k == KT - 1),
                                    )
                                for k in range(KT):
                                    nc.tensor.matmul(
                                        phg[:, 1, :],
                                        wt[:, k, eg : eg + P],
                                        xs(sc, k),
                                        start=(k == 0),
                                        stop=(k == KT - 1),
                                    )
                            pointwise(sc, db, phg, prev_o)
                            if sc == NSC - 1:
                                reload_w(db)
                    else:
                        # k-outer: each x k-slice streams 4 consecutive MMs
                        for g in range(DB // 2):
                            dbs = (2 * g, 2 * g + 1)
                            tiles = [
                                ppool.tile(
                                    [P, 2, SC], f32, tag="phg", name="phg"
                                )
                                for _ in dbs
                            ]
                            for k in range(KT):
                                xk = xs(sc, k)
                                for ti, db in enumerate(dbs):
                                    for half, base in (
                                        (0, db * P),
                                        (1, D + db * P),
                                    ):
                                        nc.tensor.matmul(
                                            tiles[ti][:, half, :],
                                            wt[:, k, base : base + P],
                                            xk,
                                            start=(k == 0),
                                            stop=(k == KT - 1),
                                        )
                            for ti, db in enumerate(dbs):
                                pointwise(sc, db, tiles[ti], prev_o)
                                if sc == NSC - 1:
                                    reload_w(db)
                    # prefetch this sc tile for the next pass (same data),
                    # one DMA per k-slice so no consumer waits on a 1MB
                    # transfer
                    if stream_inputs:
                        for k in range(KT):
                            nc.scalar.dma_start(
                                xs(sc, k),
                                xT_k[:, k, sc * SC : (sc + 1) * SC],
                            )
    nc.compile()
    return nc


def _get_nc():
    if "nc" not in _NC_CACHE:
        _NC_CACHE["nc"] = _build_bass()
    return _NC_CACHE["nc"]


def _run(in_maps, trace=False, **kw):
    from concourse import bass_utils

    nc = _get_nc()
    return bass_utils.run_bass_kernel_spmd(
        nc, in_maps, core_ids=list(range(B)), trace=trace, **kw
    )


def _make_in_maps(x, W_hg):
    import ml_dtypes

    bf = ml_dtypes.bfloat16
    x = np.asarray(x, dtype=np.float32)
    wT = np.ascontiguousarray(np.asarray(W_hg, dtype=np.float32).T)
    wT[:, D:] *= -1.0  # negated gate block: sigmoid gives a = sigmoid(-g)
    wTb = wT.astype(bf)
    return [
        {"xT": np.ascontiguousarray(x[i].T).astype(bf), "wT": wTb}
        for i in range(B)
    ]


def kernel(x, W_hg):
    res = _run(_make_in_maps(x, W_hg))
    outs = [r["out"] for r in res.results]
    return np.stack(
        [o.astype(np.float32).T for o in outs], axis=0
    )



# revision 2
# speedup vs baseline: 1.2731x; 1.2731x over previous
"""MinGRU Trainium2 kernel (v4: mixed fp8-DoubleRow/bf16 matmul).

Full-input contract: kernel(x=[8,4096,1024] f32, W_hg=[2048,1024] f32)
-> [8,4096,1024] f32.

Sharding: data-parallel over batch. Core i computes example i entirely;
W_hg is replicated.

Math (linear-space equivalent of the log-space reference):
    hg      = x @ W_hg.T ; hidden, gate = split(hg)
    a_t     = sigmoid(-gate_t)                       # = 1 - z_t
    g~_t    = max(sigmoid(hidden_t), hidden_t + 0.5) # exact identity
    h_t     = a_t h_{t-1} + (1 - a_t) g~_t
            = a_t h_{t-1} - bneg_t,  bneg_t = (a_t - 1) g~_t

Precision (error budget vs the 2e-2 harness gate, validated in CPU sim):
  - gate: all 8 k-subtiles as 4 fp8e4m3 DoubleRow matmuls (output error
    contribution ~8e-3 — sigmoid compresses gate noise).
  - hidden: k-subtiles 0..2*HP-1 as HP fp8-DR matmuls, rest bf16
    (hidden error propagates ~linearly into the scan output, so only
    half goes fp8; HP=2 -> predicted rel_fro ~1.6e-2).
  - W is pre-scaled by 32 on the host so its fp8 lands in e4m3's normal
    range (sigma_w = 1/32 would put 38% of weights in subnormals); the
    1/32 is folded into the ACT sigmoid/copy free-affine scale.

Per (sc, db) tile ([128 channels, 512 seq]):
    PE   : HP DR + (8-2*HP) bf16 matmuls -> phg[:,0,:] (hidden);
           4 DR matmuls -> phg[:,1,:] (NEGATED gate; gate weight block
           negated on host so ONE combined sigmoid yields both s and a).
    ACT  : sa = sigmoid(phg/32)        [128,1024] -> bf16 (s | a)
    ACT  : hp = phg[:,0,:]/32 + 0.5    (Copy scale+bias) -> bf16
    DVE  : gt = max(hp, s)             (tensor_tensor, bf16)
    DVE  : bneg = (a - 1.0) * gt       (stt)
    DVE  : o = scan(a, bneg, init, mult, subtract)  # o_t = a o_{t-1} - bneg
    DMA  : out[db, sc] <- o (bf16; host upcasts)

Loop order: sc OUTER, db inner -> scan carry (db, sc-1) is ready a full
sc block ahead; never on the critical path.

DMA: inputs (W, x) go on the ACT HWDGE ring, outputs on the SP ring, so
input prefetch is not FIFO-blocked behind output drains. Input tiles are
reloaded right after their last use each pass (the data is loop-invariant
in the timing loop), so the reload for pass n+1 overlaps ~7/8 of pass n.
"""

from contextlib import ExitStack

import numpy as np

B, S, D = 8, 4096, 1024
E = 2 * D
P = 128
KT = D // P  # contraction k-subtiles
DB = D // P  # output-channel pair-blocks (hidden+gate pair per block)
SC = 512  # seq chunk (PSUM bank = 512 f32)
NSC = S // SC
HP = 2  # hidden fp8 k-PAIRS (k-subtiles 0..2*HP-1 fp8, rest bf16)
KB = KT - 2 * HP  # hidden bf16 k-subtiles
WS = 32.0  # host-side W scale (folded out in ACT affine)

_NC_CACHE = {}


def _build_bass(
    repeat=1,
    loop_repeat=None,
    psum_bufs=4,
    sa_bufs=4,
    work_bufs=4,
    gt_mode="act",  # "act": hp on ACT + max on DVE; "dve": stt on DVE
    hid_pairs=HP,
    stream_inputs=1,  # 0: prologue-load only (ablation); 1: reload per pass
):
    import contextlib

    import concourse.tile as tile
    from concourse import bacc, mybir

    f32 = mybir.dt.float32
    bf16 = mybir.dt.bfloat16
    f8 = mybir.dt.float8e4
    AF = mybir.ActivationFunctionType
    OP = mybir.AluOpType
    DR = mybir.MatmulPerfMode.DoubleRow

    kb = KT - 2 * hid_pairs  # bf16 hidden k-subtiles
    dlo = 2 * hid_pairs * P  # first bf16 hidden row in D

    nc = bacc.Bacc("TRN2", debug=False)
    x8 = nc.dram_tensor("x8", [D, S], f8, kind="ExternalInput").ap()
    w8 = nc.dram_tensor("w8", [D, E], f8, kind="ExternalInput").ap()
    if kb:
        xb = nc.dram_tensor("xb", [kb * P, S], bf16, kind="ExternalInput").ap()
        wb = nc.dram_tensor("wb", [kb * P, D], bf16, kind="ExternalInput").ap()
        xb_k = xb.rearrange("(k p) s -> p k s", p=P)
        wb_k = wb.rearrange("(k p) e -> p k e", p=P)
    out = nc.dram_tensor("out", [D, S], bf16, kind="ExternalOutput").ap()

    x8_k = x8.rearrange("(k p) s -> p k s", p=P)
    w8_k = w8.rearrange("(k p) e -> p k e", p=P)

    with tile.TileContext(nc) as tc, ExitStack() as ctx:
        xpool = ctx.enter_context(tc.tile_pool(name="x", bufs=1))
        wpool = ctx.enter_context(tc.tile_pool(name="w", bufs=1))
        ppool = ctx.enter_context(
            tc.tile_pool(name="ps", bufs=psum_bufs, space="PSUM")
        )
        sapool = ctx.enter_context(tc.tile_pool(name="sa", bufs=sa_bufs))
        gpool = ctx.enter_context(tc.tile_pool(name="g", bufs=work_bufs))
        opool = ctx.enter_context(tc.tile_pool(name="o", bufs=2))

        # x tiles persist across For_i iterations; prologue-loaded once,
        # then re-loaded (same loop-invariant data) right after last use.
        x8t, xbt = [], []
        for sc in range(NSC):
            t = xpool.tile([P, KT, SC], f8, tag=f"x8{sc}", name=f"x8t{sc}")
            nc.scalar.dma_start(t[:], x8_k[:, :, sc * SC : (sc + 1) * SC])
            x8t.append(t)
            if kb:
                t = xpool.tile(
                    [P, kb, SC], bf16, tag=f"xb{sc}", name=f"xbt{sc}"
                )
                nc.scalar.dma_start(
                    t[:], xb_k[:, :, sc * SC : (sc + 1) * SC]
                )
                xbt.append(t)

        loop_cm = (
            tc.For_i(0, loop_repeat, 1)
            if loop_repeat is not None
            else contextlib.nullcontext()
        )
        # W is persistent; prologue-loaded monolithically, then per-pass
        # re-streamed in per-block slices right after each block's last
        # use, so no pass-head serialization.
        w8t = wpool.tile([P, KT, E], f8, tag="w8")
        nc.scalar.dma_start(w8t[:], w8_k)
        if kb:
            wbt = wpool.tile([P, kb, D], bf16, tag="wb")
            nc.scalar.dma_start(wbt[:], wb_k)

        with loop_cm:
            for _rep in range(repeat):

                def pointwise(sc, db, phg, prev_o):
                    # one sigmoid over both banks: s | a
                    sa = sapool.tile([P, 2, SC], bf16, tag="sa", name="sa")
                    nc.scalar.activation(
                        sa[:], phg[:], AF.Sigmoid, scale=1.0 / WS
                    )
                    s = sa[:, 0, :]
                    a = sa[:, 1, :]

                    gt = gpool.tile([P, SC], bf16, tag="gt", name="gt")
                    if gt_mode == "act":
                        hp = gpool.tile([P, SC], bf16, tag="hp", name="hp")
                        nc.scalar.activation(
                            hp[:], phg[:, 0, :], AF.Copy,
                            bias=0.5, scale=1.0 / WS,
                        )
                        nc.vector.tensor_tensor(gt[:], hp[:], s, op=OP.max)
                    else:
                        # (phg*(1/WS) + 0.5) max s is not one stt; keep a
                        # psum-side stt only for the unscaled case
                        raise NotImplementedError

                    bneg = gpool.tile([P, SC], bf16, tag="bneg", name="bneg")
                    nc.vector.scalar_tensor_tensor(
                        bneg[:], a, 1.0, gt[:],
                        op0=OP.subtract, op1=OP.mult,
                    )

                    o = opool.tile([P, SC], bf16, tag=f"o{db}", name="o")
                    init = 0.0 if sc == 0 else prev_o[db][:, SC - 1 : SC]
                    nc.vector.tensor_tensor_scan(
                        o[:], a, bneg[:], init,
                        op0=OP.mult, op1=OP.subtract,
                    )
                    prev_o[db] = o
                    nc.sync.dma_start(
                        out[db * P : db * P + P, sc * SC : (sc + 1) * SC],
                        o[:],
                    )

                def reload_w(db):
                    # re-stream this block's weight slices (same data) for
                    # the next pass, right after their last use
                    if not stream_inputs:
                        return
                    for base in (db * P, D + db * P):
                        nc.scalar.dma_start(
                            w8t[:, :, base : base + P],
                            w8_k[:, :, base : base + P],
                        )
                    if kb:
                        nc.scalar.dma_start(
                            wbt[:, :, db * P : db * P + P],
                            wb_k[:, :, db * P : db * P + P],
                        )

                prev_o = [None] * DB
                for sc in range(NSC):
                    s0 = sc * SC
                    for db in range(DB):
                        eh = db * P
                        eg = D + db * P
                        phg = ppool.tile([P, 2, SC], f32, tag="phg")
                        # hidden: fp8-DR pairs then bf16 tail
                        for j in range(hid_pairs):
                            nc.tensor.matmul(
                                phg[:, 0, :],
                                w8t[:, 2 * j : 2 * j + 2, eh : eh + P],
                                x8t[sc][:, 2 * j : 2 * j + 2, :],
                                start=(j == 0),
                                stop=(kb == 0 and j == hid_pairs - 1),
                                perf_mode=DR,
                            )
                        for k in range(kb):
                            nc.tensor.matmul(
                                phg[:, 0, :],
                                wbt[:, k, eh : eh + P],
                                xbt[sc][:, k, :],
                                start=(hid_pairs == 0 and k == 0),
                                stop=(k == kb - 1),
                            )
                        # gate: all fp8-DR
                        for j in range(KT // 2):
                            nc.tensor.matmul(
                                phg[:, 1, :],
                                w8t[:, 2 * j : 2 * j + 2, eg : eg + P],
                                x8t[sc][:, 2 * j : 2 * j + 2, :],
                                start=(j == 0),
                                stop=(j == KT // 2 - 1),
                                perf_mode=DR,
                            )
                        pointwise(sc, db, phg, prev_o)
                        if sc == NSC - 1:
                            reload_w(db)
                    # prefetch this sc tile for the next pass (same data),
                    # one DMA per k-slice so no consumer waits on a large
                    # transfer
                    if stream_inputs:
                        for k in range(KT):
                            nc.scalar.dma_start(
                                x8t[sc][:, k, :],
                                x8_k[:, k, s0 : s0 + SC],
                            )
                        for k in range(kb):
                            nc.scalar.dma_start(
                                xbt[sc][:, k, :],
                                xb_k[:, k, s0 : s0 + SC],
                            )
    nc.compile()
    return nc


def _get_nc():
    if "nc" not in _NC_CACHE:
        _NC_CACHE["nc"] = _build_bass()
    return _NC_CACHE["nc"]


def _run(in_maps, trace=False, **kw):
    from concourse import bass_utils

    nc = _get_nc()
    return bass_utils.run_bass_kernel_spmd(
        nc, in_maps, core_ids=list(range(B)), trace=trace, **kw
    )


def _make_in_maps(x, W_hg, hid_pairs=HP):
    import ml_dtypes

    bf = ml_dtypes.bfloat16
    f8 = ml_dtypes.float8_e4m3
    x = np.asarray(x, dtype=np.float32)
    wT = np.ascontiguousarray(np.asarray(W_hg, dtype=np.float32).T) * WS
    wT[:, D:] *= -1.0  # negated gate block: sigmoid gives a = sigmoid(-g)
    w8 = wT.astype(f8)
    dlo = 2 * hid_pairs * P
    wb = wT[dlo:, :D].astype(bf)
    maps = []
    for i in range(B):
        xT = np.ascontiguousarray(x[i].T)
        m = {"x8": xT.astype(f8), "w8": w8}
        if dlo < D:
            m["xb"] = xT[dlo:].astype(bf)
            m["wb"] = wb
        maps.append(m)
    return maps


def kernel(x, W_hg):
    res = _run(_make_in_maps(x, W_hg))
    outs = [r["out"] for r in res.results]
    return np.stack([o.astype(np.float32).T for o in outs], axis=0)


# revision 6
# speedup vs baseline: 1.3572x; 1.0661x over previous
"""MinGRU Trainium2 kernel (v4: mixed fp8-DoubleRow/bf16 matmul).

Full-input contract: kernel(x=[8,4096,1024] f32, W_hg=[2048,1024] f32)
-> [8,4096,1024] f32.

Sharding: data-parallel over batch. Core i computes example i entirely;
W_hg is replicated.

Math (linear-space equivalent of the log-space reference):
    hg      = x @ W_hg.T ; hidden, gate = split(hg)
    a_t     = sigmoid(-gate_t)                       # = 1 - z_t
    g~_t    = max(sigmoid(hidden_t), hidden_t + 0.5) # exact identity
    h_t     = a_t h_{t-1} + (1 - a_t) g~_t
            = a_t h_{t-1} - bneg_t,  bneg_t = (a_t - 1) g~_t

Precision (error budget vs the 2e-2 harness gate, validated in CPU sim):
  - gate: all 8 k-subtiles as 4 fp8e4m3 DoubleRow matmuls (output error
    contribution ~8e-3 — sigmoid compresses gate noise).
  - hidden: k-subtiles 0..2*HP-1 as HP fp8-DR matmuls, rest bf16
    (hidden error propagates ~linearly into the scan output, so only
    half goes fp8; HP=2 -> predicted rel_fro ~1.6e-2).
  - W is pre-scaled by 32 on the host so its fp8 lands in e4m3's normal
    range (sigma_w = 1/32 would put 38% of weights in subnormals); the
    1/32 is folded into the ACT sigmoid/copy free-affine scale.

Per (sc, db) tile ([128 channels, 512 seq]):
    PE   : HP DR + (8-2*HP) bf16 matmuls -> phg[:,0,:] (hidden);
           4 DR matmuls -> phg[:,1,:] (NEGATED gate; gate weight block
           negated on host so ONE combined sigmoid yields both s and a).
    ACT  : sa = sigmoid(phg/32)        [128,1024] -> bf16 (s | a)
    ACT  : hp = phg[:,0,:]/32 + 0.5    (Copy scale+bias) -> bf16
    DVE  : gt = max(hp, s)             (tensor_tensor, bf16)
    DVE  : bneg = (a - 1.0) * gt       (stt)
    DVE  : o = scan(a, bneg, init, mult, subtract)  # o_t = a o_{t-1} - bneg
    DMA  : out[db, sc] <- o (bf16; host upcasts)

Loop order: sc OUTER, db inner -> scan carry (db, sc-1) is ready a full
sc block ahead; never on the critical path.

DMA: inputs (W, x) go on the ACT HWDGE ring, outputs on the SP ring, so
input prefetch is not FIFO-blocked behind output drains. Input tiles are
reloaded right after their last use each pass (the data is loop-invariant
in the timing loop), so the reload for pass n+1 overlaps ~7/8 of pass n.
"""

from contextlib import ExitStack

import numpy as np

B, S, D = 8, 4096, 1024
E = 2 * D
P = 128
KT = D // P  # contraction k-subtiles
DB = D // P  # output-channel pair-blocks (hidden+gate pair per block)
SC = 512  # seq chunk (PSUM bank = 512 f32)
NSC = S // SC
HP = 2  # hidden fp8 k-PAIRS (k-subtiles 0..2*HP-1 fp8, rest bf16)
KB = KT - 2 * HP  # hidden bf16 k-subtiles
WS = 32.0  # host-side W scale (folded out in ACT affine)

_NC_CACHE = {}


def _build_bass(
    repeat=1,
    loop_repeat=None,
    psum_bufs=4,
    sa_bufs=4,
    work_bufs=4,
    gt_mode="act",  # "act": hp on ACT + max on DVE; "dve": stt on DVE
    hid_pairs=HP,
    stream_inputs=1,  # 0: prologue-load only (ablation); 1: reload per pass
    stages=5,  # ablation: 0=PE only, 1=+sig, 2=+hp, 3=+gt, 4=+bneg, 5=full
    mm_group="split",  # "split": hidDR,hidBF,gateDR; "dr_first": DR then BF
):
    import contextlib

    import concourse.tile as tile
    from concourse import bacc, mybir

    f32 = mybir.dt.float32
    bf16 = mybir.dt.bfloat16
    f8 = mybir.dt.float8e4
    AF = mybir.ActivationFunctionType
    OP = mybir.AluOpType
    DR = mybir.MatmulPerfMode.DoubleRow

    kb = KT - 2 * hid_pairs  # bf16 hidden k-subtiles
    dlo = 2 * hid_pairs * P  # first bf16 hidden row in D

    nc = bacc.Bacc("TRN2", debug=False)
    x8 = nc.dram_tensor("x8", [D, S], f8, kind="ExternalInput").ap()
    w8 = nc.dram_tensor("w8", [D, E], f8, kind="ExternalInput").ap()
    if kb:
        xb = nc.dram_tensor("xb", [kb * P, S], bf16, kind="ExternalInput").ap()
        wb = nc.dram_tensor("wb", [kb * P, D], bf16, kind="ExternalInput").ap()
        xb_k = xb.rearrange("(k p) s -> p k s", p=P)
        wb_k = wb.rearrange("(k p) e -> p k e", p=P)
    out = nc.dram_tensor("out", [D, S], bf16, kind="ExternalOutput").ap()

    x8_k = x8.rearrange("(k p) s -> p k s", p=P)
    w8_k = w8.rearrange("(k p) e -> p k e", p=P)

    with tile.TileContext(nc) as tc, ExitStack() as ctx:
        xpool = ctx.enter_context(tc.tile_pool(name="x", bufs=1))
        wpool = ctx.enter_context(tc.tile_pool(name="w", bufs=1))
        ppool = ctx.enter_context(
            tc.tile_pool(name="ps", bufs=psum_bufs, space="PSUM")
        )
        sapool = ctx.enter_context(tc.tile_pool(name="sa", bufs=sa_bufs))
        gpool = ctx.enter_context(tc.tile_pool(name="g", bufs=work_bufs))
        opool = ctx.enter_context(tc.tile_pool(name="o", bufs=2))

        # x tiles persist across For_i iterations; prologue-loaded once,
        # then re-loaded (same loop-invariant data) right after last use.
        x8t, xbt = [], []
        for sc in range(NSC):
            t = xpool.tile([P, KT, SC], f8, tag=f"x8{sc}", name=f"x8t{sc}")
            nc.scalar.dma_start(t[:], x8_k[:, :, sc * SC : (sc + 1) * SC])
            x8t.append(t)
            if kb:
                t = xpool.tile(
                    [P, kb, SC], bf16, tag=f"xb{sc}", name=f"xbt{sc}"
                )
                nc.scalar.dma_start(
                    t[:], xb_k[:, :, sc * SC : (sc + 1) * SC]
                )
                xbt.append(t)

        loop_cm = (
            tc.For_i(0, loop_repeat, 1)
            if loop_repeat is not None
            else contextlib.nullcontext()
        )
        # W is persistent; prologue-loaded monolithically, then per-pass
        # re-streamed in per-block slices right after each block's last
        # use, so no pass-head serialization.
        w8t = wpool.tile([P, KT, E], f8, tag="w8")
        nc.scalar.dma_start(w8t[:], w8_k)
        if kb:
            wbt = wpool.tile([P, kb, D], bf16, tag="wb")
            nc.scalar.dma_start(wbt[:], wb_k)

        with loop_cm:
            for _rep in range(repeat):

                def pointwise(sc, db, phg, prev_o):
                    if stages < 1:
                        return
                    # one sigmoid over both banks: s | a
                    sa = sapool.tile([P, 2, SC], bf16, tag="sa", name="sa")
                    nc.scalar.activation(
                        sa[:], phg[:], AF.Sigmoid, scale=1.0 / WS
                    )
                    s = sa[:, 0, :]
                    a = sa[:, 1, :]

                    if stages < 2:
                        return
                    gt = gpool.tile([P, SC], bf16, tag="gt", name="gt")
                    if gt_mode == "act":
                        hp = gpool.tile([P, SC], bf16, tag="hp", name="hp")
                        nc.scalar.activation(
                            hp[:], phg[:, 0, :], AF.Copy,
                            bias=0.5, scale=1.0 / WS,
                        )
                        if stages < 3:
                            return
                        nc.vector.tensor_tensor(gt[:], hp[:], s, op=OP.max)
                    else:
                        # (phg*(1/WS) + 0.5) max s is not one stt; keep a
                        # psum-side stt only for the unscaled case
                        raise NotImplementedError

                    if stages < 4:
                        return
                    bneg = gpool.tile([P, SC], bf16, tag="bneg", name="bneg")
                    nc.vector.scalar_tensor_tensor(
                        bneg[:], a, 1.0, gt[:],
                        op0=OP.subtract, op1=OP.mult,
                    )

                    if stages < 5:
                        return
                    o = opool.tile([P, SC], bf16, tag=f"o{db}", name="o")
                    init = 0.0 if sc == 0 else prev_o[db][:, SC - 1 : SC]
                    nc.vector.tensor_tensor_scan(
                        o[:], a, bneg[:], init,
                        op0=OP.mult, op1=OP.subtract,
                    )
                    prev_o[db] = o
                    nc.sync.dma_start(
                        out[db * P : db * P + P, sc * SC : (sc + 1) * SC],
                        o[:],
                    )

                def reload_w(db):
                    # re-stream this block's weight slices (same data) for
                    # the next pass, right after their last use
                    if not stream_inputs:
                        return
                    for base in (db * P, D + db * P):
                        nc.scalar.dma_start(
                            w8t[:, :, base : base + P],
                            w8_k[:, :, base : base + P],
                        )
                    if kb:
                        nc.scalar.dma_start(
                            wbt[:, :, db * P : db * P + P],
                            wb_k[:, :, db * P : db * P + P],
                        )

                def mm_hid_dr(phg, db):
                    eh = db * P
                    for j in range(hid_pairs):
                        nc.tensor.matmul(
                            phg[:, 0, :],
                            w8t[:, 2 * j : 2 * j + 2, eh : eh + P],
                            x8t[mm_sc][:, 2 * j : 2 * j + 2, :],
                            start=(j == 0),
                            stop=(kb == 0 and j == hid_pairs - 1),
                            perf_mode=DR,
                        )

                def mm_hid_bf(phg, db):
                    eh = db * P
                    for k in range(kb):
                        nc.tensor.matmul(
                            phg[:, 0, :],
                            wbt[:, k, eh : eh + P],
                            xbt[mm_sc][:, k, :],
                            start=(hid_pairs == 0 and k == 0),
                            stop=(k == kb - 1),
                        )

                def mm_gate_dr(phg, db):
                    eg = D + db * P
                    for j in range(KT // 2):
                        nc.tensor.matmul(
                            phg[:, 1, :],
                            w8t[:, 2 * j : 2 * j + 2, eg : eg + P],
                            x8t[mm_sc][:, 2 * j : 2 * j + 2, :],
                            start=(j == 0),
                            stop=(j == KT // 2 - 1),
                            perf_mode=DR,
                        )

                prev_o = [None] * DB
                for sc in range(NSC):
                    s0 = sc * SC
                    mm_sc = sc
                    if mm_group == "phase2":
                        # batch DR and BF phases over pairs of db blocks:
                        # one DR->BF mode switch per 2 tiles instead of 2
                        # per tile
                        for g in range(DB // 2):
                            dbs = (2 * g, 2 * g + 1)
                            tiles = [
                                ppool.tile([P, 2, SC], f32, tag="phg")
                                for _ in dbs
                            ]
                            for t, db in zip(tiles, dbs):
                                mm_hid_dr(t, db)
                                mm_gate_dr(t, db)
                            for t, db in zip(tiles, dbs):
                                mm_hid_bf(t, db)
                            for t, db in zip(tiles, dbs):
                                pointwise(sc, db, t, prev_o)
                                if sc == NSC - 1:
                                    reload_w(db)
                    else:
                        for db in range(DB):
                            phg = ppool.tile([P, 2, SC], f32, tag="phg")
                            if mm_group == "dr_first":
                                mm_hid_dr(phg, db)
                                mm_gate_dr(phg, db)
                                mm_hid_bf(phg, db)
                            else:
                                mm_hid_dr(phg, db)
                                mm_hid_bf(phg, db)
                                mm_gate_dr(phg, db)
                            pointwise(sc, db, phg, prev_o)
                            if sc == NSC - 1:
                                reload_w(db)
                    # prefetch this sc tile for the next pass (same data),
                    # one DMA per k-slice so no consumer waits on a large
                    # transfer
                    if stream_inputs:
                        for k in range(KT):
                            nc.scalar.dma_start(
                                x8t[sc][:, k, :],
                                x8_k[:, k, s0 : s0 + SC],
                            )
                        for k in range(kb):
                            nc.scalar.dma_start(
                                xbt[sc][:, k, :],
                                xb_k[:, k, s0 : s0 + SC],
                            )
    nc.compile()
    return nc


def _get_nc():
    if "nc" not in _NC_CACHE:
        _NC_CACHE["nc"] = _build_bass()
    return _NC_CACHE["nc"]


def _run(in_maps, trace=False, **kw):
    from concourse import bass_utils

    nc = _get_nc()
    return bass_utils.run_bass_kernel_spmd(
        nc, in_maps, core_ids=list(range(B)), trace=trace, **kw
    )


def _make_in_maps(x, W_hg, hid_pairs=HP):
    import ml_dtypes

    bf = ml_dtypes.bfloat16
    f8 = ml_dtypes.float8_e4m3
    x = np.asarray(x, dtype=np.float32)
    wT = np.ascontiguousarray(np.asarray(W_hg, dtype=np.float32).T) * WS
    wT[:, D:] *= -1.0  # negated gate block: sigmoid gives a = sigmoid(-g)
    w8 = wT.astype(f8)
    dlo = 2 * hid_pairs * P
    wb = wT[dlo:, :D].astype(bf)
    maps = []
    for i in range(B):
        xT = np.ascontiguousarray(x[i].T)
        m = {"x8": xT.astype(f8), "w8": w8}
        if dlo < D:
            m["xb"] = xT[dlo:].astype(bf)
            m["wb"] = wb
        maps.append(m)
    return maps


def kernel(x, W_hg):
    res = _run(_make_in_maps(x, W_hg))
    outs = [r["out"] for r in res.results]
    return np.stack([o.astype(np.float32).T for o in outs], axis=0)


# revision 10
# speedup vs baseline: 1.6259x; 1.1979x over previous
"""MinGRU Trainium2 kernel (v4: mixed fp8-DoubleRow/bf16 matmul).

Full-input contract: kernel(x=[8,4096,1024] f32, W_hg=[2048,1024] f32)
-> [8,4096,1024] f32.

Sharding: data-parallel over batch. Core i computes example i entirely;
W_hg is replicated.

Math (linear-space equivalent of the log-space reference):
    hg      = x @ W_hg.T ; hidden, gate = split(hg)
    a_t     = sigmoid(-gate_t)                       # = 1 - z_t
    g~_t    = max(sigmoid(hidden_t), hidden_t + 0.5) # exact identity
    h_t     = a_t h_{t-1} + (1 - a_t) g~_t
            = a_t h_{t-1} - bneg_t,  bneg_t = (a_t - 1) g~_t

Precision (error budget vs the 2e-2 harness gate, validated in CPU sim):
  - gate: all 8 k-subtiles as 4 fp8e4m3 DoubleRow matmuls (output error
    contribution ~8e-3 — sigmoid compresses gate noise).
  - hidden: k-subtiles 0..2*HP-1 as HP fp8-DR matmuls, rest bf16
    (hidden error propagates ~linearly into the scan output, so only
    half goes fp8; HP=2 -> predicted rel_fro ~1.6e-2).
  - W is pre-scaled by 32 on the host so its fp8 lands in e4m3's normal
    range (sigma_w = 1/32 would put 38% of weights in subnormals); the
    1/32 is folded into the ACT sigmoid/copy free-affine scale.

Per (sc, db) tile ([128 channels, 512 seq]):
    PE   : HP DR + (8-2*HP) bf16 matmuls -> phg[:,0,:] (hidden);
           4 DR matmuls -> phg[:,1,:] (NEGATED gate; gate weight block
           negated on host so ONE combined sigmoid yields both s and a).
    ACT  : sa = sigmoid(phg/32)        [128,1024] -> bf16 (s | a)
    ACT  : hp = phg[:,0,:]/32 + 0.5    (Copy scale+bias) -> bf16
    DVE  : gt = max(hp, s)             (tensor_tensor, bf16)
    DVE  : bneg = (a - 1.0) * gt       (stt)
    DVE  : o = scan(a, bneg, init, mult, subtract)  # o_t = a o_{t-1} - bneg
    DMA  : out[db, sc] <- o (bf16; host upcasts)

Loop order: sc OUTER, db inner -> scan carry (db, sc-1) is ready a full
sc block ahead; never on the critical path.

DMA: inputs (W, x) go on the ACT HWDGE ring, outputs on the SP ring, so
input prefetch is not FIFO-blocked behind output drains. Input tiles are
reloaded right after their last use each pass (the data is loop-invariant
in the timing loop), so the reload for pass n+1 overlaps ~7/8 of pass n.
"""

from contextlib import ExitStack

import numpy as np

B, S, D = 8, 4096, 1024
E = 2 * D
P = 128
KT = D // P  # contraction k-subtiles
DB = D // P  # output-channel pair-blocks (hidden+gate pair per block)
SC = 512  # seq chunk (PSUM bank = 512 f32)
NSC = S // SC
HP = 2  # hidden fp8 k-PAIRS (k-subtiles 0..2*HP-1 fp8, rest bf16)
KB = KT - 2 * HP  # hidden bf16 k-subtiles
WS = 32.0  # host-side W scale (folded out in ACT affine)

_NC_CACHE = {}


def _build_bass(
    repeat=1,
    loop_repeat=None,
    psum_bufs=4,
    sa_bufs=6,
    work_bufs=6,
    gt_mode="act",  # "act": hp on ACT + max on DVE; "dve": stt on DVE
    hid_pairs=HP,
    stream_inputs=1,  # 0: prologue-load only (ablation); 1: reload per pass
    stages=5,  # ablation: 0=PE only, 1=+sig, 2=+hp, 3=+gt, 4=+bneg, 5=full
    mm_group="phase2",  # phase2: batch DR/BF mode phases over db pairs
):
    import contextlib

    import concourse.tile as tile
    from concourse import bacc, mybir

    f32 = mybir.dt.float32
    bf16 = mybir.dt.bfloat16
    f8 = mybir.dt.float8e4
    AF = mybir.ActivationFunctionType
    OP = mybir.AluOpType
    DR = mybir.MatmulPerfMode.DoubleRow

    kb = KT - 2 * hid_pairs  # bf16 hidden k-subtiles
    dlo = 2 * hid_pairs * P  # first bf16 hidden row in D

    nc = bacc.Bacc("TRN2", debug=False)
    x8 = nc.dram_tensor("x8", [D, S], f8, kind="ExternalInput").ap()
    w8 = nc.dram_tensor("w8", [D, E], f8, kind="ExternalInput").ap()
    if kb:
        xb = nc.dram_tensor("xb", [kb * P, S], bf16, kind="ExternalInput").ap()
        wb = nc.dram_tensor("wb", [kb * P, D], bf16, kind="ExternalInput").ap()
        xb_k = xb.rearrange("(k p) s -> p k s", p=P)
        wb_k = wb.rearrange("(k p) e -> p k e", p=P)
    out = nc.dram_tensor("out", [D, S], bf16, kind="ExternalOutput").ap()

    x8_k = x8.rearrange("(k p) s -> p k s", p=P)
    w8_k = w8.rearrange("(k p) e -> p k e", p=P)

    with tile.TileContext(nc) as tc, ExitStack() as ctx:
        xpool = ctx.enter_context(tc.tile_pool(name="x", bufs=1))
        wpool = ctx.enter_context(tc.tile_pool(name="w", bufs=1))
        ppool = ctx.enter_context(
            tc.tile_pool(name="ps", bufs=psum_bufs, space="PSUM")
        )
        sapool = ctx.enter_context(tc.tile_pool(name="sa", bufs=sa_bufs))
        gpool = ctx.enter_context(tc.tile_pool(name="g", bufs=work_bufs))
        opool = ctx.enter_context(tc.tile_pool(name="o", bufs=2))

        # x tiles persist across For_i iterations; prologue-loaded once,
        # then re-loaded (same loop-invariant data) right after last use.
        x8t, xbt = [], []
        for sc in range(NSC):
            t = xpool.tile([P, KT, SC], f8, tag=f"x8{sc}", name=f"x8t{sc}")
            nc.scalar.dma_start(t[:], x8_k[:, :, sc * SC : (sc + 1) * SC])
            x8t.append(t)
            if kb:
                t = xpool.tile(
                    [P, kb, SC], bf16, tag=f"xb{sc}", name=f"xbt{sc}"
                )
                nc.scalar.dma_start(
                    t[:], xb_k[:, :, sc * SC : (sc + 1) * SC]
                )
                xbt.append(t)

        loop_cm = (
            tc.For_i(0, loop_repeat, 1)
            if loop_repeat is not None
            else contextlib.nullcontext()
        )
        # W is persistent; prologue-loaded monolithically, then per-pass
        # re-streamed in per-block slices right after each block's last
        # use, so no pass-head serialization.
        w8t = wpool.tile([P, KT, E], f8, tag="w8")
        nc.scalar.dma_start(w8t[:], w8_k)
        if kb:
            wbt = wpool.tile([P, kb, D], bf16, tag="wb")
            nc.scalar.dma_start(wbt[:], wb_k)

        with loop_cm:
            for _rep in range(repeat):

                def pointwise(sc, db, phg, prev_o):
                    if stages < 1:
                        return
                    # one sigmoid over both banks: s | a
                    sa = sapool.tile([P, 2, SC], bf16, tag="sa", name="sa")
                    nc.scalar.activation(
                        sa[:], phg[:], AF.Sigmoid, scale=1.0 / WS
                    )
                    s = sa[:, 0, :]
                    a = sa[:, 1, :]

                    if stages < 2:
                        return
                    gt = gpool.tile([P, SC], bf16, tag="gt", name="gt")
                    if gt_mode == "act":
                        hp = gpool.tile([P, SC], bf16, tag="hp", name="hp")
                        nc.scalar.activation(
                            hp[:], phg[:, 0, :], AF.Copy,
                            bias=0.5, scale=1.0 / WS,
                        )
                        if stages < 3:
                            return
                        nc.vector.tensor_tensor(gt[:], hp[:], s, op=OP.max)
                    else:
                        # (phg*(1/WS) + 0.5) max s is not one stt; keep a
                        # psum-side stt only for the unscaled case
                        raise NotImplementedError

                    if stages < 4:
                        return
                    bneg = gpool.tile([P, SC], bf16, tag="bneg", name="bneg")
                    nc.vector.scalar_tensor_tensor(
                        bneg[:], a, 1.0, gt[:],
                        op0=OP.subtract, op1=OP.mult,
                    )

                    if stages < 5:
                        return
                    o = opool.tile([P, SC], bf16, tag=f"o{db}", name="o")
                    init = 0.0 if sc == 0 else prev_o[db][:, SC - 1 : SC]
                    nc.vector.tensor_tensor_scan(
                        o[:], a, bneg[:], init,
                        op0=OP.mult, op1=OP.subtract,
                    )
                    prev_o[db] = o
                    nc.sync.dma_start(
                        out[db * P : db * P + P, sc * SC : (sc + 1) * SC],
                        o[:],
                    )

                def reload_w(db):
                    # re-stream this block's weight slices (same data) for
                    # the next pass, right after their last use
                    if not stream_inputs:
                        return
                    for base in (db * P, D + db * P):
                        nc.scalar.dma_start(
                            w8t[:, :, base : base + P],
                            w8_k[:, :, base : base + P],
                        )
                    if kb:
                        nc.scalar.dma_start(
                            wbt[:, :, db * P : db * P + P],
                            wb_k[:, :, db * P : db * P + P],
                        )

                def mm_hid_dr(phg, db):
                    eh = db * P
                    for j in range(hid_pairs):
                        nc.tensor.matmul(
                            phg[:, 0, :],
                            w8t[:, 2 * j : 2 * j + 2, eh : eh + P],
                            x8t[mm_sc][:, 2 * j : 2 * j + 2, :],
                            start=(j == 0),
                            stop=(kb == 0 and j == hid_pairs - 1),
                            perf_mode=DR,
                        )

                def mm_hid_bf(phg, db):
                    eh = db * P
                    for k in range(kb):
                        nc.tensor.matmul(
                            phg[:, 0, :],
                            wbt[:, k, eh : eh + P],
                            xbt[mm_sc][:, k, :],
                            start=(hid_pairs == 0 and k == 0),
                            stop=(k == kb - 1),
                        )

                def mm_gate_dr(phg, db):
                    eg = D + db * P
                    for j in range(KT // 2):
                        nc.tensor.matmul(
                            phg[:, 1, :],
                            w8t[:, 2 * j : 2 * j + 2, eg : eg + P],
                            x8t[mm_sc][:, 2 * j : 2 * j + 2, :],
                            start=(j == 0),
                            stop=(j == KT // 2 - 1),
                            perf_mode=DR,
                        )

                prev_o = [None] * DB
                for sc in range(NSC):
                    s0 = sc * SC
                    mm_sc = sc
                    if mm_group in ("phase2", "phase2i"):
                        # batch DR and BF phases over pairs of db blocks:
                        # one DR->BF mode switch per 2 tiles instead of 2
                        # per tile
                        for g in range(DB // 2):
                            dbs = (2 * g, 2 * g + 1)
                            tiles = [
                                ppool.tile(
                                    [P, 2, SC], f32, tag="phg", name="phg"
                                )
                                for _ in dbs
                            ]
                            for t, db in zip(tiles, dbs):
                                mm_hid_dr(t, db)
                                mm_gate_dr(t, db)
                            if mm_group == "phase2i":
                                for t, db in zip(tiles, dbs):
                                    mm_hid_bf(t, db)
                                    pointwise(sc, db, t, prev_o)
                                    if sc == NSC - 1:
                                        reload_w(db)
                            else:
                                for t, db in zip(tiles, dbs):
                                    mm_hid_bf(t, db)
                                for t, db in zip(tiles, dbs):
                                    pointwise(sc, db, t, prev_o)
                                    if sc == NSC - 1:
                                        reload_w(db)
                    else:
                        for db in range(DB):
                            phg = ppool.tile([P, 2, SC], f32, tag="phg")
                            if mm_group == "dr_first":
                                mm_hid_dr(phg, db)
                                mm_gate_dr(phg, db)
                                mm_hid_bf(phg, db)
                            else:
                                mm_hid_dr(phg, db)
                                mm_hid_bf(phg, db)
                                mm_gate_dr(phg, db)
                            pointwise(sc, db, phg, prev_o)
                            if sc == NSC - 1:
                                reload_w(db)
                    # prefetch this sc tile for the next pass (same data),
                    # one DMA per k-slice so no consumer waits on a large
                    # transfer
                    if stream_inputs:
                        for k in range(KT):
                            nc.scalar.dma_start(
                                x8t[sc][:, k, :],
                                x8_k[:, k, s0 : s0 + SC],
                            )
                        for k in range(kb):
                            nc.scalar.dma_start(
                                xbt[sc][:, k, :],
                                xb_k[:, k, s0 : s0 + SC],
                            )
    nc.compile()
    return nc


def _get_nc():
    if "nc" not in _NC_CACHE:
        _NC_CACHE["nc"] = _build_bass()
    return _NC_CACHE["nc"]


def _run(in_maps, trace=False, **kw):
    from concourse import bass_utils

    nc = _get_nc()
    return bass_utils.run_bass_kernel_spmd(
        nc, in_maps, core_ids=list(range(B)), trace=trace, **kw
    )


def _make_in_maps(x, W_hg, hid_pairs=None):
    import ml_dtypes

    if hid_pairs is None:
        hid_pairs = HP

    bf = ml_dtypes.bfloat16
    f8 = ml_dtypes.float8_e4m3
    x = np.asarray(x, dtype=np.float32)
    wT = np.ascontiguousarray(np.asarray(W_hg, dtype=np.float32).T) * WS
    wT[:, D:] *= -1.0  # negated gate block: sigmoid gives a = sigmoid(-g)
    w8 = wT.astype(f8)
    dlo = 2 * hid_pairs * P
    wb = wT[dlo:, :D].astype(bf)
    maps = []
    for i in range(B):
        xT = np.ascontiguousarray(x[i].T)
        m = {"x8": xT.astype(f8), "w8": w8}
        if dlo < D:
            m["xb"] = xT[dlo:].astype(bf)
            m["wb"] = wb
        maps.append(m)
    return maps


def kernel(x, W_hg):
    res = _run(_make_in_maps(x, W_hg))
    outs = [r["out"] for r in res.results]
    return np.stack([o.astype(np.float32).T for o in outs], axis=0)


# revision 13
# speedup vs baseline: 1.9539x; 1.2017x over previous
"""MinGRU Trainium2 kernel (v4: mixed fp8-DoubleRow/bf16 matmul).

Full-input contract: kernel(x=[8,4096,1024] f32, W_hg=[2048,1024] f32)
-> [8,4096,1024] f32.

Sharding: data-parallel over batch. Core i computes example i entirely;
W_hg is replicated.

Math (linear-space equivalent of the log-space reference):
    hg      = x @ W_hg.T ; hidden, gate = split(hg)
    a_t     = sigmoid(-gate_t)                       # = 1 - z_t
    g~_t    = max(sigmoid(hidden_t), hidden_t + 0.5) # exact identity
    h_t     = a_t h_{t-1} + (1 - a_t) g~_t
            = a_t h_{t-1} - bneg_t,  bneg_t = (a_t - 1) g~_t

Precision (error budget vs the 2e-2 harness gate, validated in CPU sim):
  - gate: all 8 k-subtiles as 4 fp8e4m3 DoubleRow matmuls (output error
    contribution ~8e-3 — sigmoid compresses gate noise).
  - hidden: k-subtiles 0..2*HP-1 as HP fp8-DR matmuls, rest bf16
    (hidden error propagates ~linearly into the scan output, so only
    half goes fp8; HP=2 -> predicted rel_fro ~1.6e-2).
  - W is pre-scaled by 32 on the host so its fp8 lands in e4m3's normal
    range (sigma_w = 1/32 would put 38% of weights in subnormals); the
    1/32 is folded into the ACT sigmoid/copy free-affine scale.

Per (sc, db) tile ([128 channels, 512 seq]):
    PE   : HP DR + (8-2*HP) bf16 matmuls -> phg[:,0,:] (hidden);
           4 DR matmuls -> phg[:,1,:] (NEGATED gate; gate weight block
           negated on host so ONE combined sigmoid yields both s and a).
    ACT  : sa = sigmoid(phg/32)        [128,1024] -> bf16 (s | a)
    ACT  : hp = phg[:,0,:]/32 + 0.5    (Copy scale+bias) -> bf16
    DVE  : gt = max(hp, s)             (tensor_tensor, bf16)
    DVE  : bneg = (a - 1.0) * gt       (stt)
    DVE  : o = scan(a, bneg, init, mult, subtract)  # o_t = a o_{t-1} - bneg
    DMA  : out[db, sc] <- o (bf16; host upcasts)

Loop order: sc OUTER, db inner -> scan carry (db, sc-1) is ready a full
sc block ahead; never on the critical path.

DMA: prologue input loads on the ACT HWDGE ring, per-pass input reloads
on the GPSIMD SWDGE ring (their sem waits would stall sigmoid dispatch
in the ACT FIFO), outputs on the SP ring. Input tiles are reloaded right
after their last use each pass (the data is loop-invariant in the
timing loop), so the reload for pass n+1 overlaps ~7/8 of pass n.

Matmul order ("phase2"): per pair of db tiles, all 12 fp8-DR matmuls
then all 8 bf16 matmuls — one perf-mode switch per 2 tiles. Interleaving
modes per tile costs ~300-500 ns/tile; batched reaches the 216 ns/MM
silicon rate (N/2.4GHz + NX dispatch).
"""

from contextlib import ExitStack

import numpy as np

B, S, D = 8, 4096, 1024
E = 2 * D
P = 128
KT = D // P  # contraction k-subtiles
DB = D // P  # output-channel pair-blocks (hidden+gate pair per block)
SC = 512  # seq chunk (PSUM bank = 512 f32)
NSC = S // SC
HP = 2  # hidden fp8 k-PAIRS (k-subtiles 0..2*HP-1 fp8, rest bf16)
KB = KT - 2 * HP  # hidden bf16 k-subtiles
WS = 32.0  # host-side W scale (folded out in ACT affine)

_NC_CACHE = {}


def _build_bass(
    repeat=1,
    loop_repeat=None,
    psum_bufs=4,
    sa_bufs=6,
    work_bufs=6,
    gt_mode="act",  # "act": hp on ACT + max on DVE; "dve": stt on DVE
    hid_pairs=HP,
    stream_inputs=1,  # 0: prologue-load only (ablation); 1: reload per pass
    stages=5,  # ablation: 0=PE only, 1=+sig, 2=+hp, 3=+gt, 4=+bneg, 5=full
    mm_group="phase2",  # phase2: batch DR/BF mode phases over db pairs
    reload_ring="gpsimd",  # SWDGE queue: keeps reload DMA waits out of
    # the ACT FIFO, which would stall sigmoid/hp dispatch (-14us measured)
):
    import contextlib

    import concourse.tile as tile
    from concourse import bacc, mybir

    f32 = mybir.dt.float32
    bf16 = mybir.dt.bfloat16
    f8 = mybir.dt.float8e4
    AF = mybir.ActivationFunctionType
    OP = mybir.AluOpType
    DR = mybir.MatmulPerfMode.DoubleRow

    kb = KT - 2 * hid_pairs  # bf16 hidden k-subtiles
    dlo = 2 * hid_pairs * P  # first bf16 hidden row in D

    nc = bacc.Bacc("TRN2", debug=False)
    x8 = nc.dram_tensor("x8", [D, S], f8, kind="ExternalInput").ap()
    w8 = nc.dram_tensor("w8", [D, E], f8, kind="ExternalInput").ap()
    if kb:
        xb = nc.dram_tensor("xb", [kb * P, S], bf16, kind="ExternalInput").ap()
        wb = nc.dram_tensor("wb", [kb * P, D], bf16, kind="ExternalInput").ap()
        xb_k = xb.rearrange("(k p) s -> p k s", p=P)
        wb_k = wb.rearrange("(k p) e -> p k e", p=P)
    out = nc.dram_tensor("out", [D, S], bf16, kind="ExternalOutput").ap()

    x8_k = x8.rearrange("(k p) s -> p k s", p=P)
    w8_k = w8.rearrange("(k p) e -> p k e", p=P)

    with tile.TileContext(nc) as tc, ExitStack() as ctx:
        xpool = ctx.enter_context(tc.tile_pool(name="x", bufs=1))
        wpool = ctx.enter_context(tc.tile_pool(name="w", bufs=1))
        ppool = ctx.enter_context(
            tc.tile_pool(name="ps", bufs=psum_bufs, space="PSUM")
        )
        sapool = ctx.enter_context(tc.tile_pool(name="sa", bufs=sa_bufs))
        gpool = ctx.enter_context(tc.tile_pool(name="g", bufs=work_bufs))
        opool = ctx.enter_context(tc.tile_pool(name="o", bufs=2))

        # x tiles persist across For_i iterations; prologue-loaded once,
        # then re-loaded (same loop-invariant data) right after last use.
        x8t, xbt = [], []
        for sc in range(NSC):
            t = xpool.tile([P, KT, SC], f8, tag=f"x8{sc}", name=f"x8t{sc}")
            nc.scalar.dma_start(t[:], x8_k[:, :, sc * SC : (sc + 1) * SC])
            x8t.append(t)
            if kb:
                t = xpool.tile(
                    [P, kb, SC], bf16, tag=f"xb{sc}", name=f"xbt{sc}"
                )
                nc.scalar.dma_start(
                    t[:], xb_k[:, :, sc * SC : (sc + 1) * SC]
                )
                xbt.append(t)

        loop_cm = (
            tc.For_i(0, loop_repeat, 1)
            if loop_repeat is not None
            else contextlib.nullcontext()
        )
        # W is persistent; prologue-loaded monolithically, then per-pass
        # re-streamed in per-block slices right after each block's last
        # use, so no pass-head serialization.
        w8t = wpool.tile([P, KT, E], f8, tag="w8")
        nc.scalar.dma_start(w8t[:], w8_k)
        if kb:
            wbt = wpool.tile([P, kb, D], bf16, tag="wb")
            nc.scalar.dma_start(wbt[:], wb_k)

        with loop_cm:
            for _rep in range(repeat):

                def pointwise(sc, db, phg, prev_o):
                    if stages < 1:
                        return
                    # one sigmoid over both banks: s | a
                    sa = sapool.tile([P, 2, SC], bf16, tag="sa", name="sa")
                    nc.scalar.activation(
                        sa[:], phg[:], AF.Sigmoid, scale=1.0 / WS
                    )
                    s = sa[:, 0, :]
                    a = sa[:, 1, :]

                    if stages < 2:
                        return
                    gt = gpool.tile([P, SC], bf16, tag="gt", name="gt")
                    if gt_mode == "act":
                        hp = gpool.tile([P, SC], bf16, tag="hp", name="hp")
                        nc.scalar.activation(
                            hp[:], phg[:, 0, :], AF.Copy,
                            bias=0.5, scale=1.0 / WS,
                        )
                        if stages < 3:
                            return
                        nc.vector.tensor_tensor(gt[:], hp[:], s, op=OP.max)
                    else:
                        # (phg*(1/WS) + 0.5) max s is not one stt; keep a
                        # psum-side stt only for the unscaled case
                        raise NotImplementedError

                    if stages < 4:
                        return
                    bneg = gpool.tile([P, SC], bf16, tag="bneg", name="bneg")
                    nc.vector.scalar_tensor_tensor(
                        bneg[:], a, 1.0, gt[:],
                        op0=OP.subtract, op1=OP.mult,
                    )

                    if stages < 5:
                        return
                    o = opool.tile([P, SC], bf16, tag=f"o{db}", name="o")
                    init = 0.0 if sc == 0 else prev_o[db][:, SC - 1 : SC]
                    nc.vector.tensor_tensor_scan(
                        o[:], a, bneg[:], init,
                        op0=OP.mult, op1=OP.subtract,
                    )
                    prev_o[db] = o
                    nc.sync.dma_start(
                        out[db * P : db * P + P, sc * SC : (sc + 1) * SC],
                        o[:],
                    )

                rdma = getattr(nc, reload_ring)

                def reload_w(db):
                    # re-stream this block's weight slices (same data) for
                    # the next pass, right after their last use
                    if not stream_inputs:
                        return
                    for base in (db * P, D + db * P):
                        rdma.dma_start(
                            w8t[:, :, base : base + P],
                            w8_k[:, :, base : base + P],
                        )
                    if kb:
                        rdma.dma_start(
                            wbt[:, :, db * P : db * P + P],
                            wb_k[:, :, db * P : db * P + P],
                        )

                def mm_hid_dr(phg, db):
                    eh = db * P
                    for j in range(hid_pairs):
                        nc.tensor.matmul(
                            phg[:, 0, :],
                            w8t[:, 2 * j : 2 * j + 2, eh : eh + P],
                            x8t[mm_sc][:, 2 * j : 2 * j + 2, :],
                            start=(j == 0),
                            stop=(kb == 0 and j == hid_pairs - 1),
                            perf_mode=DR,
                        )

                def mm_hid_bf(phg, db):
                    eh = db * P
                    for k in range(kb):
                        nc.tensor.matmul(
                            phg[:, 0, :],
                            wbt[:, k, eh : eh + P],
                            xbt[mm_sc][:, k, :],
                            start=(hid_pairs == 0 and k == 0),
                            stop=(k == kb - 1),
                        )

                def mm_gate_dr(phg, db):
                    eg = D + db * P
                    for j in range(KT // 2):
                        nc.tensor.matmul(
                            phg[:, 1, :],
                            w8t[:, 2 * j : 2 * j + 2, eg : eg + P],
                            x8t[mm_sc][:, 2 * j : 2 * j + 2, :],
                            start=(j == 0),
                            stop=(j == KT // 2 - 1),
                            perf_mode=DR,
                        )

                prev_o = [None] * DB
                for sc in range(NSC):
                    s0 = sc * SC
                    mm_sc = sc
                    if mm_group in ("phase2", "phase2i"):
                        # batch DR and BF phases over pairs of db blocks:
                        # one DR->BF mode switch per 2 tiles instead of 2
                        # per tile
                        for g in range(DB // 2):
                            dbs = (2 * g, 2 * g + 1)
                            tiles = [
                                ppool.tile(
                                    [P, 2, SC], f32, tag="phg", name="phg"
                                )
                                for _ in dbs
                            ]
                            for t, db in zip(tiles, dbs):
                                mm_hid_dr(t, db)
                                mm_gate_dr(t, db)
                            if mm_group == "phase2i":
                                for t, db in zip(tiles, dbs):
                                    mm_hid_bf(t, db)
                                    pointwise(sc, db, t, prev_o)
                                    if sc == NSC - 1:
                                        reload_w(db)
                            else:
                                for t, db in zip(tiles, dbs):
                                    mm_hid_bf(t, db)
                                for t, db in zip(tiles, dbs):
                                    pointwise(sc, db, t, prev_o)
                                    if sc == NSC - 1:
                                        reload_w(db)
                    else:
                        for db in range(DB):
                            phg = ppool.tile([P, 2, SC], f32, tag="phg")
                            if mm_group == "dr_first":
                                mm_hid_dr(phg, db)
                                mm_gate_dr(phg, db)
                                mm_hid_bf(phg, db)
                            else:
                                mm_hid_dr(phg, db)
                                mm_hid_bf(phg, db)
                                mm_gate_dr(phg, db)
                            pointwise(sc, db, phg, prev_o)
                            if sc == NSC - 1:
                                reload_w(db)
                    # prefetch this sc tile for the next pass (same data),
                    # one DMA per k-slice so no consumer waits on a large
                    # transfer
                    if stream_inputs:
                        for k in range(KT):
                            rdma.dma_start(
                                x8t[sc][:, k, :],
                                x8_k[:, k, s0 : s0 + SC],
                            )
                        for k in range(kb):
                            rdma.dma_start(
                                xbt[sc][:, k, :],
                                xb_k[:, k, s0 : s0 + SC],
                            )
    nc.compile()
    return nc


def _get_nc():
    if "nc" not in _NC_CACHE:
        _NC_CACHE["nc"] = _build_bass()
    return _NC_CACHE["nc"]


def _run(in_maps, trace=False, **kw):
    from concourse import bass_utils

    nc = _get_nc()
    return bass_utils.run_bass_kernel_spmd(
        nc, in_maps, core_ids=list(range(B)), trace=trace, **kw
    )


def _make_in_maps(x, W_hg, hid_pairs=None):
    import ml_dtypes

    if hid_pairs is None:
        hid_pairs = HP

    bf = ml_dtypes.bfloat16
    f8 = ml_dtypes.float8_e4m3
    x = np.asarray(x, dtype=np.float32)
    wT = np.ascontiguousarray(np.asarray(W_hg, dtype=np.float32).T) * WS
    wT[:, D:] *= -1.0  # negated gate block: sigmoid gives a = sigmoid(-g)
    w8 = wT.astype(f8)
    dlo = 2 * hid_pairs * P
    wb = wT[dlo:, :D].astype(bf)
    maps = []
    for i in range(B):
        xT = np.ascontiguousarray(x[i].T)
        m = {"x8": xT.astype(f8), "w8": w8}
        if dlo < D:
            m["xb"] = xT[dlo:].astype(bf)
            m["wb"] = wb
        maps.append(m)
    return maps


def kernel(x, W_hg):
    res = _run(_make_in_maps(x, W_hg))
    outs = [r["out"] for r in res.results]
    return np.stack([o.astype(np.float32).T for o in outs], axis=0)


# revision 15
# speedup vs baseline: 2.4559x; 1.2569x over previous
"""MinGRU Trainium2 kernel (v4: mixed fp8-DoubleRow/bf16 matmul).

Full-input contract: kernel(x=[8,4096,1024] f32, W_hg=[2048,1024] f32)
-> [8,4096,1024] f32.

Sharding: data-parallel over batch. Core i computes example i entirely;
W_hg is replicated.

Math (linear-space equivalent of the log-space reference):
    hg      = x @ W_hg.T ; hidden, gate = split(hg)
    a_t     = sigmoid(-gate_t)                       # = 1 - z_t
    g~_t    = max(sigmoid(hidden_t), hidden_t + 0.5) # exact identity
    h_t     = a_t h_{t-1} + (1 - a_t) g~_t
            = a_t h_{t-1} - bneg_t,  bneg_t = (a_t - 1) g~_t

Precision (error budget vs the 2e-2 harness gate, validated in CPU sim):
  - gate: all 8 k-subtiles as 4 fp8e4m3 DoubleRow matmuls (output error
    contribution ~8e-3 — sigmoid compresses gate noise).
  - hidden: k-subtiles 0..2*HP-1 as HP fp8-DR matmuls, rest bf16
    (hidden error propagates ~linearly into the scan output, so part
    stays bf16; HP=3 -> rel_fro 1.857e-2 HW-measured, 7% under the
    gate; HP=2 -> 1.594e-2 if more margin is ever needed).
  - W is pre-scaled by 32 on the host so its fp8 lands in e4m3's normal
    range (sigma_w = 1/32 would put 38% of weights in subnormals); the
    1/32 is folded into the ACT sigmoid/copy free-affine scale.

Per (sc, db) tile ([128 channels, 512 seq]):
    PE   : HP DR + (8-2*HP) bf16 matmuls -> phg[:,0,:] (hidden);
           4 DR matmuls -> phg[:,1,:] (NEGATED gate; gate weight block
           negated on host so ONE combined sigmoid yields both s and a).
    ACT  : sa = sigmoid(phg/32)        [128,1024] -> bf16 (s | a)
    ACT  : hp = phg[:,0,:]/32 + 0.5    (Copy scale+bias) -> bf16
    DVE  : gt = max(hp, s)             (tensor_tensor, bf16)
    DVE  : bneg = (a - 1.0) * gt       (stt)
    DVE  : o = scan(a, bneg, init, mult, subtract)  # o_t = a o_{t-1} - bneg
    DMA  : out[db, sc] <- o (bf16; host upcasts)

Loop order: sc OUTER, db inner -> scan carry (db, sc-1) is ready a full
sc block ahead; never on the critical path.

DMA: prologue input loads on the ACT HWDGE ring, per-pass input reloads
on the GPSIMD SWDGE ring (their sem waits would stall sigmoid dispatch
in the ACT FIFO), outputs on the SP ring. Input tiles are reloaded right
after their last use each pass (the data is loop-invariant in the
timing loop), so the reload for pass n+1 overlaps ~7/8 of pass n.

Matmul order ("phase2"): per pair of db tiles, all 12 fp8-DR matmuls
then all 8 bf16 matmuls — one perf-mode switch per 2 tiles. Interleaving
modes per tile costs ~300-500 ns/tile; batched reaches the 216 ns/MM
silicon rate (N/2.4GHz + NX dispatch).
"""

from contextlib import ExitStack

import numpy as np

B, S, D = 8, 4096, 1024
E = 2 * D
P = 128
KT = D // P  # contraction k-subtiles
DB = D // P  # output-channel pair-blocks (hidden+gate pair per block)
SC = 512  # seq chunk (PSUM bank = 512 f32)
NSC = S // SC
HP = 3  # hidden fp8 k-PAIRS (k-subtiles 0..2*HP-1 fp8, rest bf16)
KB = KT - 2 * HP  # hidden bf16 k-subtiles
WS = 32.0  # host-side W scale (folded out in ACT affine)

_NC_CACHE = {}


def _build_bass(
    repeat=1,
    loop_repeat=None,
    psum_bufs=4,
    sa_bufs=6,
    work_bufs=6,
    gt_mode="act",  # "act": hp on ACT + max on DVE; "dve": stt on DVE
    hid_pairs=HP,
    stream_inputs=1,  # 0: prologue-load only (ablation); 1: reload per pass
    stages=5,  # ablation: 0=PE only, 1=+sig, 2=+hp, 3=+gt, 4=+bneg, 5=full
    mm_group="phase2",  # phase2: batch DR/BF mode phases over db pairs
    reload_ring="gpsimd",  # SWDGE queue: keeps reload DMA waits out of
    # the ACT FIFO, which would stall sigmoid/hp dispatch (-14us measured)
):
    import contextlib

    import concourse.tile as tile
    from concourse import bacc, mybir

    f32 = mybir.dt.float32
    bf16 = mybir.dt.bfloat16
    f8 = mybir.dt.float8e4
    AF = mybir.ActivationFunctionType
    OP = mybir.AluOpType
    DR = mybir.MatmulPerfMode.DoubleRow

    kb = KT - 2 * hid_pairs  # bf16 hidden k-subtiles
    dlo = 2 * hid_pairs * P  # first bf16 hidden row in D

    nc = bacc.Bacc("TRN2", debug=False)
    x8 = nc.dram_tensor("x8", [D, S], f8, kind="ExternalInput").ap()
    w8 = nc.dram_tensor("w8", [D, E], f8, kind="ExternalInput").ap()
    if kb:
        xb = nc.dram_tensor("xb", [kb * P, S], bf16, kind="ExternalInput").ap()
        wb = nc.dram_tensor("wb", [kb * P, D], bf16, kind="ExternalInput").ap()
        xb_k = xb.rearrange("(k p) s -> p k s", p=P)
        wb_k = wb.rearrange("(k p) e -> p k e", p=P)
    out = nc.dram_tensor("out", [D, S], bf16, kind="ExternalOutput").ap()

    x8_k = x8.rearrange("(k p) s -> p k s", p=P)
    w8_k = w8.rearrange("(k p) e -> p k e", p=P)

    with tile.TileContext(nc) as tc, ExitStack() as ctx:
        xpool = ctx.enter_context(tc.tile_pool(name="x", bufs=1))
        wpool = ctx.enter_context(tc.tile_pool(name="w", bufs=1))
        ppool = ctx.enter_context(
            tc.tile_pool(name="ps", bufs=psum_bufs, space="PSUM")
        )
        sapool = ctx.enter_context(tc.tile_pool(name="sa", bufs=sa_bufs))
        gpool = ctx.enter_context(tc.tile_pool(name="g", bufs=work_bufs))
        opool = ctx.enter_context(tc.tile_pool(name="o", bufs=2))

        # x tiles persist across For_i iterations; prologue-loaded once,
        # then re-loaded (same loop-invariant data) right after last use.
        x8t, xbt = [], []
        for sc in range(NSC):
            t = xpool.tile([P, KT, SC], f8, tag=f"x8{sc}", name=f"x8t{sc}")
            nc.scalar.dma_start(t[:], x8_k[:, :, sc * SC : (sc + 1) * SC])
            x8t.append(t)
            if kb:
                t = xpool.tile(
                    [P, kb, SC], bf16, tag=f"xb{sc}", name=f"xbt{sc}"
                )
                nc.scalar.dma_start(
                    t[:], xb_k[:, :, sc * SC : (sc + 1) * SC]
                )
                xbt.append(t)

        loop_cm = (
            tc.For_i(0, loop_repeat, 1)
            if loop_repeat is not None
            else contextlib.nullcontext()
        )
        # W is persistent; prologue-loaded monolithically, then per-pass
        # re-streamed in per-block slices right after each block's last
        # use, so no pass-head serialization.
        w8t = wpool.tile([P, KT, E], f8, tag="w8")
        nc.scalar.dma_start(w8t[:], w8_k)
        if kb:
            wbt = wpool.tile([P, kb, D], bf16, tag="wb")
            nc.scalar.dma_start(wbt[:], wb_k)

        with loop_cm:
            for _rep in range(repeat):

                def pointwise(sc, db, phg, prev_o):
                    if stages < 1:
                        return
                    # one sigmoid over both banks: s | a
                    sa = sapool.tile([P, 2, SC], bf16, tag="sa", name="sa")
                    nc.scalar.activation(
                        sa[:], phg[:], AF.Sigmoid, scale=1.0 / WS
                    )
                    s = sa[:, 0, :]
                    a = sa[:, 1, :]

                    if stages < 2:
                        return
                    gt = gpool.tile([P, SC], bf16, tag="gt", name="gt")
                    if gt_mode == "act":
                        hp = gpool.tile([P, SC], bf16, tag="hp", name="hp")
                        nc.scalar.activation(
                            hp[:], phg[:, 0, :], AF.Copy,
                            bias=0.5, scale=1.0 / WS,
                        )
                        if stages < 3:
                            return
                        nc.vector.tensor_tensor(gt[:], hp[:], s, op=OP.max)
                    else:
                        # (phg*(1/WS) + 0.5) max s is not one stt; keep a
                        # psum-side stt only for the unscaled case
                        raise NotImplementedError

                    if stages < 4:
                        return
                    bneg = gpool.tile([P, SC], bf16, tag="bneg", name="bneg")
                    nc.vector.scalar_tensor_tensor(
                        bneg[:], a, 1.0, gt[:],
                        op0=OP.subtract, op1=OP.mult,
                    )

                    if stages < 5:
                        return
                    o = opool.tile([P, SC], bf16, tag=f"o{db}", name="o")
                    init = 0.0 if sc == 0 else prev_o[db][:, SC - 1 : SC]
                    nc.vector.tensor_tensor_scan(
                        o[:], a, bneg[:], init,
                        op0=OP.mult, op1=OP.subtract,
                    )
                    prev_o[db] = o
                    nc.sync.dma_start(
                        out[db * P : db * P + P, sc * SC : (sc + 1) * SC],
                        o[:],
                    )

                rdma = getattr(nc, reload_ring)

                def reload_w(db):
                    # re-stream this block's weight slices (same data) for
                    # the next pass, right after their last use
                    if not stream_inputs:
                        return
                    for base in (db * P, D + db * P):
                        rdma.dma_start(
                            w8t[:, :, base : base + P],
                            w8_k[:, :, base : base + P],
                        )
                    if kb:
                        rdma.dma_start(
                            wbt[:, :, db * P : db * P + P],
                            wb_k[:, :, db * P : db * P + P],
                        )

                def mm_hid_dr(phg, db):
                    eh = db * P
                    for j in range(hid_pairs):
                        nc.tensor.matmul(
                            phg[:, 0, :],
                            w8t[:, 2 * j : 2 * j + 2, eh : eh + P],
                            x8t[mm_sc][:, 2 * j : 2 * j + 2, :],
                            start=(j == 0),
                            stop=(kb == 0 and j == hid_pairs - 1),
                            perf_mode=DR,
                        )

                def mm_hid_bf(phg, db):
                    eh = db * P
                    for k in range(kb):
                        nc.tensor.matmul(
                            phg[:, 0, :],
                            wbt[:, k, eh : eh + P],
                            xbt[mm_sc][:, k, :],
                            start=(hid_pairs == 0 and k == 0),
                            stop=(k == kb - 1),
                        )

                def mm_gate_dr(phg, db):
                    eg = D + db * P
                    for j in range(KT // 2):
                        nc.tensor.matmul(
                            phg[:, 1, :],
                            w8t[:, 2 * j : 2 * j + 2, eg : eg + P],
                            x8t[mm_sc][:, 2 * j : 2 * j + 2, :],
                            start=(j == 0),
                            stop=(j == KT // 2 - 1),
                            perf_mode=DR,
                        )

                prev_o = [None] * DB
                for sc in range(NSC):
                    s0 = sc * SC
                    mm_sc = sc
                    if mm_group in ("phase2", "phase2i"):
                        # batch DR and BF phases over pairs of db blocks:
                        # one DR->BF mode switch per 2 tiles instead of 2
                        # per tile
                        for g in range(DB // 2):
                            dbs = (2 * g, 2 * g + 1)
                            tiles = [
                                ppool.tile(
                                    [P, 2, SC], f32, tag="phg", name="phg"
                                )
                                for _ in dbs
                            ]
                            for t, db in zip(tiles, dbs):
                                mm_hid_dr(t, db)
                                mm_gate_dr(t, db)
                            if mm_group == "phase2i":
                                for t, db in zip(tiles, dbs):
                                    mm_hid_bf(t, db)
                                    pointwise(sc, db, t, prev_o)
                                    if sc == NSC - 1:
                                        reload_w(db)
                            else:
                                for t, db in zip(tiles, dbs):
                                    mm_hid_bf(t, db)
                                for t, db in zip(tiles, dbs):
                                    pointwise(sc, db, t, prev_o)
                                    if sc == NSC - 1:
                                        reload_w(db)
                    else:
                        for db in range(DB):
                            phg = ppool.tile([P, 2, SC], f32, tag="phg")
                            if mm_group == "dr_first":
                                mm_hid_dr(phg, db)
                                mm_gate_dr(phg, db)
                                mm_hid_bf(phg, db)
                            else:
                                mm_hid_dr(phg, db)
                                mm_hid_bf(phg, db)
                                mm_gate_dr(phg, db)
                            pointwise(sc, db, phg, prev_o)
                            if sc == NSC - 1:
                                reload_w(db)
                    # prefetch this sc tile for the next pass (same data),
                    # one DMA per k-slice so no consumer waits on a large
                    # transfer
                    if stream_inputs:
                        for k in range(KT):
                            rdma.dma_start(
                                x8t[sc][:, k, :],
                                x8_k[:, k, s0 : s0 + SC],
                            )
                        for k in range(kb):
                            rdma.dma_start(
                                xbt[sc][:, k, :],
                                xb_k[:, k, s0 : s0 + SC],
                            )
    nc.compile()
    return nc


def _get_nc():
    if "nc" not in _NC_CACHE:
        _NC_CACHE["nc"] = _build_bass()
    return _NC_CACHE["nc"]


def _run(in_maps, trace=False, **kw):
    from concourse import bass_utils

    nc = _get_nc()
    return bass_utils.run_bass_kernel_spmd(
        nc, in_maps, core_ids=list(range(B)), trace=trace, **kw
    )


def _make_in_maps(x, W_hg, hid_pairs=None):
    import ml_dtypes

    if hid_pairs is None:
        hid_pairs = HP

    bf = ml_dtypes.bfloat16
    f8 = ml_dtypes.float8_e4m3
    x = np.asarray(x, dtype=np.float32)
    wT = np.ascontiguousarray(np.asarray(W_hg, dtype=np.float32).T) * WS
    wT[:, D:] *= -1.0  # negated gate block: sigmoid gives a = sigmoid(-g)
    w8 = wT.astype(f8)
    dlo = 2 * hid_pairs * P
    wb = wT[dlo:, :D].astype(bf)
    maps = []
    for i in range(B):
        xT = np.ascontiguousarray(x[i].T)
        m = {"x8": xT.astype(f8), "w8": w8}
        if dlo < D:
            m["xb"] = xT[dlo:].astype(bf)
            m["wb"] = wb
        maps.append(m)
    return maps


def kernel(x, W_hg):
    res = _run(_make_in_maps(x, W_hg))
    outs = [r["out"] for r in res.results]
    return np.stack([o.astype(np.float32).T for o in outs], axis=0)
